# revision 37
# baseline (speedup 1.0000x reference)
"""Trainium2 Bass kernel for nn_MoEALU (soft ripple-carry byte adder), v3.

Restructured math (validated in sim.py against the jax reference):
  - nibble sums: segmented sums of the 256-wide byte distribution per pos.
  - softmax1 kept UNNORMALIZED (te = exp(100(c - max))); the normalizer
    kappa_s = 1/(sum te_a * sum te_b) = 1/sum_m u_raw[m] is folded into the
    17 conv outputs afterwards.
  - cyclic conv u[m] = sum_i xa_i xb_{(m-i)%16} via a doubled-xb buffer
    (stride [+1 m, -1 i] AP); z15 == u[15].
  - Z1 = 1 - sum_i xa_i p[15-i] where p = prefix sums of xb (one scan).
  - carry chain: softmax over 2 == sigmoid => gamma' = sig(100(d + e*gamma)),
    d = Z1-Z0 = 1-2W, e = 2*z15. At temp 100 gamma saturates to {0,1}, so the
    recurrence linearizes EXACTLY (validated): gamma' = v0 + (v1-v0)*gamma
    with v0 = sig(100 d), v1 = sig(100(d+e)) => one tensor_tensor_scan with
    per-tile reset via zeroed b-coefficient.
  - s-logits: s = u + (rot1(u) - u)*gamma_in.
  - output softmax factorizes: softmax_256(100(sh_i + sl_j)) =
    softmax_16(100 sh) (x) softmax_16(100 sl): two 16-wide softmaxes and an
    outer product per byte; chain softmaxes use the fixed offset exp(100v-100).
fp16: te / conv products / u storage / A / final outer + DMA-out. exp outputs
eh/e2 stay fp32 (fp16 underflows for near-flat dists); per-side r2 folds
(r2h*r2l overflows fp32).

Schedule: phase A software-pipelined per tile; carry chain + phase C run per
16-tile half, with phase C emitted in 8-tile chunks x 4 stages interleaved
into the phase-A stream so DVE never stalls on ACT exps.

Sharding: pure data parallel over batch, 8 cores x 4096 rows.
"""

import numpy as np

B_FULL = 32768
N_CORES = 8
B_CORE = B_FULL // N_CORES  # 4096
P = 128
NT = B_CORE // P  # 32 tiles
HT = NT // 2  # tiles per half
CT = 8  # tiles per phase-C chunk

_BUILT = None


def _build():
    import concourse.bass as bass
    import concourse.bacc as bacc
    import concourse.mybir as mybir
    import concourse.tile as tile

    f32 = mybir.dt.float32
    f16 = mybir.dt.float16
    AF = mybir.ActivationFunctionType
    AX = mybir.AxisListType
    OP = mybir.AluOpType

    nc = bacc.Bacc("TRN2", target_bir_lowering=False, debug=False)
    a_d = nc.dram_tensor("a", [B_CORE, 4, 256], f32, kind="ExternalInput")
    b_d = nc.dram_tensor("b", [B_CORE, 4, 256], f32, kind="ExternalInput")
    out_d = nc.dram_tensor("out", [B_CORE, 4, 256], f16, kind="ExternalOutput")

    def ap(base_ap, off, dims):
        part = base_ap.ap[0]
        return bass.AP(base_ap.tensor, base_ap.offset + off,
                       [list(part)] + [list(d) for d in dims])

    with tile.TileContext(nc) as tc:
        with (
            tc.tile_pool(name="persist", bufs=1) as pp,
            tc.tile_pool(name="pin", bufs=2) as pin,
            tc.tile_pool(name="pa", bufs=2) as pa,
            tc.tile_pool(name="pa1", bufs=2) as pa1,
            tc.tile_pool(name="pc", bufs=2) as pc,
            tc.tile_pool(name="prep", bufs=2) as prep,
            tc.tile_pool(name="pout", bufs=3) as pout,
        ):
            # ---------------- persistent tensors ----------------
            u_all = pp.tile([P, NT, 8, 18], f16, tag="u_all")
            d_all = pp.tile([P, NT, 8], f32, tag="d_all")
            e_all = pp.tile([P, NT, 8], f32, tag="e_all")
            dpe = pp.tile([P, NT, 8], f32, tag="dpe")
            v0 = pp.tile([P, NT, 8], f32, tag="v0")
            v1 = pp.tile([P, NT, 8], f32, tag="v1")
            bco = pp.tile([P, NT, 8], f32, tag="bco")
            gg = pp.tile([P, 257], f32, tag="gg")
            g16 = pp.tile([P, NT, 8, 16], f16, tag="g16")
            nb100 = pp.tile([P, 1], f32, tag="nb100")
            nc.gpsimd.memset(nb100[:], -100.0)
            nc.gpsimd.memset(ap(bco[:], 0, [[8, NT]]), 0.0)

            a_v = a_d.ap().rearrange("(n p) f g -> n p (f g)", p=P)
            b_v = b_d.ap().rearrange("(n p) f g -> n p (f g)", p=P)
            o_v = out_d.ap().rearrange("(n p) f g -> n p (f g)", p=P)

            # ----- phase A stage 1 (2-tile super-tile): load+sums+max ----
            # ab layout [tensor, t, 1024] so (tensor,t) merges to stride 1024
            def phase_a1(u):
                ab = pin.tile([P, 2, 2, 1024], f32, tag="ab")
                order = ((0, 0), (0, 1), (1, 0), (1, 1)) if u == 0 else \
                    ((0, 0), (1, 0), (0, 1), (1, 1))
                for T2, t in order:
                    nc.sync.dma_start(
                        ap(ab[:], 2048 * T2 + 1024 * t, [[1, 1024]]),
                        (a_v if T2 == 0 else b_v)[2 * u + t])
                # c_all [tensor, t, 8 stages, 16]
                c_all = pa1.tile([P, 512], f32, tag="c_all")
                lo1 = pa.tile([P, 4, 4, 16, 8], f32, tag="lo1")
                if u == 0:
                    # split per tensor-half so compute starts after 2 DMAs
                    for T2 in range(2):
                        nc.vector.tensor_reduce(
                            ap(c_all[:], 256 * T2 + 16,
                               [[128, 2], [32, 4], [1, 16]]),
                            ap(ab[:], 2048 * T2,
                               [[1024, 2], [256, 4], [16, 16], [1, 16]]),
                            axis=AX.X, op=OP.add)
                        nc.gpsimd.tensor_add(
                            ap(lo1[:], 1024 * T2,
                               [[512, 2], [128, 4], [8, 16], [1, 8]]),
                            ap(ab[:], 2048 * T2,
                               [[1024, 2], [256, 4], [1, 16], [16, 8]]),
                            ap(ab[:], 2048 * T2 + 128,
                               [[1024, 2], [256, 4], [1, 16], [16, 8]]))
                else:
                    nc.vector.tensor_reduce(
                        ap(c_all[:], 16, [[128, 4], [32, 4], [1, 16]]),
                        ap(ab[:], 0, [[1024, 4], [256, 4], [16, 16], [1, 16]]),
                        axis=AX.X, op=OP.add)
                    nc.gpsimd.tensor_add(
                        lo1[:],
                        ap(ab[:], 0, [[1024, 4], [256, 4], [1, 16], [16, 8]]),
                        ap(ab[:], 128, [[1024, 4], [256, 4], [1, 16], [16, 8]]))
                lo2 = pa.tile([P, 4, 4, 16, 4], f32, tag="lo2")
                nc.gpsimd.tensor_add(
                    lo2[:],
                    ap(lo1[:], 0, [[512, 4], [128, 4], [8, 16], [1, 4]]),
                    ap(lo1[:], 4, [[512, 4], [128, 4], [8, 16], [1, 4]]))
                lo3 = pa.tile([P, 4, 4, 16, 2], f32, tag="lo3")
                nc.gpsimd.tensor_add(
                    lo3[:],
                    ap(lo2[:], 0, [[256, 4], [64, 4], [4, 16], [1, 2]]),
                    ap(lo2[:], 2, [[256, 4], [64, 4], [4, 16], [1, 2]]))
                nc.gpsimd.tensor_add(
                    ap(c_all[:], 0, [[128, 4], [32, 4], [1, 16]]),
                    ap(lo3[:], 0, [[128, 4], [32, 4], [2, 16]]),
                    ap(lo3[:], 1, [[128, 4], [32, 4], [2, 16]]))
                m16 = pa1.tile([P, 32], f32, tag="m16")
                nc.vector.tensor_reduce(
                    m16[:], c_all[:].rearrange("p (g e) -> p g e", g=32),
                    axis=AX.X, op=OP.max)
                return c_all, m16

            # ----- phase A stage 2 (2-tile super-tile) -----
            def phase_a2(u, c_all, m16):
                ts = pa.tile([P, 512], f32, tag="ts")
                nc.gpsimd.tensor_sub(
                    ts[:].rearrange("p (g e) -> p g e", g=32),
                    c_all[:].rearrange("p (g e) -> p g e", g=32),
                    ap(m16[:], 0, [[1, 32], [0, 16]]))
                # T [tensor, t, s, 16]: b-half contiguous at 256 for the scan
                T = pa.tile([P, 2, 2, 8, 16], f16, tag="T")
                nc.scalar.activation(
                    T[:].rearrange("p a b c d -> p (a b c d)"),
                    ts[:], AF.Exp, scale=100.0)
                xbd = pa.tile([P, 2, 8, 32], f16, tag="xbd")
                nc.scalar.copy(
                    ap(xbd[:], 0, [[256, 2], [32, 8], [1, 16]]),
                    ap(T[:], 256, [[128, 2], [16, 8], [1, 16]]))
                nc.scalar.copy(
                    ap(xbd[:], 16, [[256, 2], [32, 8], [1, 16]]),
                    ap(T[:], 256, [[128, 2], [16, 8], [1, 16]]))
                S = pa.tile([P, 260], f32, tag="S")
                nc.gpsimd.memset(ap(S[:], 0, [[1, 1]]), 0.0)
                nc.vector.tensor_tensor_scan(
                    ap(S[:], 1, [[1, 256]]),
                    ap(T[:], 256, [[1, 256]]),
                    ap(T[:], 256, [[1, 256]]),
                    0.0, OP.add, OP.bypass)
                p16 = pa.tile([P, 2, 8, 16], f16, tag="p16")
                nc.vector.tensor_sub(
                    p16[:],
                    ap(S[:], 1, [[16, 16], [1, 16]]),
                    ap(S[:], 0, [[16, 16], [0, 16]]))
                q = pa.tile([P, 2, 8, 17, 16], f16, tag="q")
                nc.vector.tensor_mul(
                    ap(q[:], 0, [[2176, 2], [272, 8], [16, 16], [1, 16]]),
                    ap(T[:], 0, [[128, 2], [16, 8], [0, 16], [1, 16]]),
                    ap(xbd[:], 16, [[256, 2], [32, 8], [1, 16], [-1, 16]]))
                nc.vector.tensor_mul(
                    ap(q[:], 256, [[2176, 2], [272, 8], [1, 16]]),
                    ap(T[:], 0, [[128, 2], [16, 8], [1, 16]]),
                    ap(p16[:], 15, [[128, 2], [16, 8], [-1, 16]]))
                qt1 = pa.tile([P, 2, 8, 17, 8], f16, tag="qt1")
                nc.vector.tensor_add(
                    qt1[:],
                    ap(q[:], 0, [[2176, 2], [272, 8], [16, 17], [1, 8]]),
                    ap(q[:], 8, [[2176, 2], [272, 8], [16, 17], [1, 8]]))
                qt2 = pa.tile([P, 2, 8, 17, 4], f16, tag="qt2")
                nc.vector.tensor_add(
                    qt2[:],
                    ap(qt1[:], 0, [[1088, 2], [136, 8], [8, 17], [1, 4]]),
                    ap(qt1[:], 4, [[1088, 2], [136, 8], [8, 17], [1, 4]]))
                qt3 = pa.tile([P, 2, 8, 17, 2], f16, tag="qt3")
                nc.vector.tensor_add(
                    qt3[:],
                    ap(qt2[:], 0, [[544, 2], [68, 8], [4, 17], [1, 2]]),
                    ap(qt2[:], 2, [[544, 2], [68, 8], [4, 17], [1, 2]]))
                qr = pa.tile([P, 2, 8, 17], f16, tag="qr")
                nc.vector.tensor_add(
                    qr[:],
                    ap(qt3[:], 0, [[272, 2], [34, 8], [2, 17]]),
                    ap(qt3[:], 1, [[272, 2], [34, 8], [2, 17]]))
                Su = pa.tile([P, 16], f32, tag="Su")
                nc.vector.tensor_reduce(
                    Su[:], ap(qr[:], 0, [[17, 16], [1, 16]]),
                    axis=AX.X, op=OP.add)
                rk = pa.tile([P, 16], f32, tag="rk")
                nc.vector.reciprocal(rk[:], Su[:])
                nc.gpsimd.tensor_mul(
                    ap(u_all[:], 288 * u + 1, [[144, 2], [18, 8], [1, 16]]),
                    ap(qr[:], 0, [[136, 2], [17, 8], [1, 16]]),
                    ap(rk[:], 0, [[8, 2], [1, 8], [0, 16]]))
                nc.gpsimd.tensor_mul(
                    ap(u_all[:], 288 * u, [[144, 2], [18, 8]]),
                    ap(qr[:], 15, [[136, 2], [17, 8]]),
                    ap(rk[:], 0, [[8, 2], [1, 8]]))
                t8 = pa.tile([P, 16], f32, tag="t8")
                nc.gpsimd.tensor_mul(
                    t8[:], ap(qr[:], 16, [[136, 2], [17, 8]]),
                    ap(rk[:], 0, [[8, 2], [1, 8]]))
                nc.vector.tensor_scalar(
                    ap(d_all[:], 16 * u, [[1, 16]]), t8[:],
                    -2.0, 1.0, op0=OP.mult, op1=OP.add)
                nc.vector.tensor_scalar_mul(
                    ap(e_all[:], 16 * u, [[1, 16]]),
                    ap(u_all[:], 288 * u, [[144, 2], [18, 8]]), 2.0)

            # --------- carry chain for tiles [t0, t0+nt) ---------
            def chain_q(t0, nt=8):
                o = 8 * t0
                dsl = ap(d_all[:], o, [[1, 8 * nt]])
                esl = ap(e_all[:], o, [[1, 8 * nt]])
                psl = ap(dpe[:], o, [[1, 8 * nt]])
                v0s = ap(v0[:], o, [[1, 8 * nt]])
                v1s = ap(v1[:], o, [[1, 8 * nt]])
                nc.gpsimd.tensor_add(psl, dsl, esl)
                nc.scalar.activation(v0s, dsl, AF.Sigmoid, scale=100.0)
                nc.scalar.activation(v1s, psl, AF.Sigmoid, scale=100.0)
                nc.gpsimd.tensor_sub(
                    ap(bco[:], o + 1, [[8, nt], [1, 7]]),
                    ap(v1[:], o + 1, [[8, nt], [1, 7]]),
                    ap(v0[:], o + 1, [[8, nt], [1, 7]]))
                nc.vector.tensor_tensor_scan(
                    ap(gg[:], o + 1, [[1, 8 * nt]]),
                    ap(bco[:], o, [[1, 8 * nt]]),
                    ap(v0[:], o, [[1, 8 * nt]]),
                    0.0, OP.mult, OP.add)
                # gg[o+8t] slots hold only discarded stage-7 carry-outs
                nc.gpsimd.memset(ap(gg[:], o, [[8, nt]]), 0.0)
                nc.scalar.copy(
                    ap(g16[:], 128 * t0, [[128, nt], [16, 8], [1, 16]]),
                    ap(gg[:], o, [[8, nt], [1, 8], [0, 16]]))

            # --------- phase C chunk stages (CT=8 tiles each) ---------
            def pc_s1(t0, ct):
                o = 144 * t0
                u_sl = ap(u_all[:], o + 1, [[144, ct], [18, 8], [1, 16]])
                rot_sl = ap(u_all[:], o, [[144, ct], [18, 8], [1, 16]])
                dlt = pc.tile([P, ct, 8, 16], f16, tag="dlt")
                tb = pc.tile([P, ct, 8, 16], f16, tag="tb")
                sb = pc.tile([P, ct, 8, 16], f16, tag="sb")
                eh = pc.tile([P, ct, 8, 16], f32, tag="eh")
                nc.vector.tensor_sub(dlt[:], rot_sl, u_sl)
                nc.vector.tensor_mul(
                    tb[:], dlt[:],
                    ap(g16[:], 128 * t0, [[128, ct], [16, 8], [1, 16]]))
                nc.vector.tensor_add(sb[:], u_sl, tb[:])
                nc.scalar.activation(eh[:], sb[:], AF.Exp,
                                     bias=nb100[:], scale=100.0)
                return eh

            def pc_s2(t0, ct, eh):
                ns = pc.tile([P, ct, 8], f32, tag="ns")
                r1 = pc.tile([P, ct, 8], f32, tag="r1")
                A16 = pc.tile([P, ct, 8, 16], f16, tag="A16")
                e2 = pc.tile([P, ct, 8, 16], f32, tag="e2")
                nc.vector.tensor_reduce(
                    ns[:], eh[:].rearrange("p a b e -> p (a b) e"),
                    axis=AX.X, op=OP.add)
                nc.vector.reciprocal(r1[:], ns[:])
                nc.gpsimd.tensor_mul(
                    A16[:], eh[:],
                    ap(r1[:], 0, [[8, ct], [1, 8], [0, 16]]))
                nc.scalar.activation(e2[:], A16[:], AF.Exp,
                                     bias=nb100[:], scale=100.0)
                return e2

            def pc_s3(t0, ct, e2, tail=False):
                s2 = pc.tile([P, ct, 8], f32, tag="s2")
                r2 = pc.tile([P, ct, 8], f32, tag="r2")
                e2h = pc.tile([P, ct, 4, 16], f16, tag="e2h")
                e2l = pc.tile([P, ct, 4, 16], f16, tag="e2l")
                nc.vector.tensor_reduce(
                    s2[:], e2[:].rearrange("p a b e -> p (a b) e"),
                    axis=AX.X, op=OP.add)
                nc.vector.reciprocal(r2[:], s2[:])
                nc.vector.tensor_mul(
                    e2h[:],
                    ap(e2[:], 16, [[128, ct], [32, 4], [1, 16]]),
                    ap(r2[:], 1, [[8, ct], [2, 4], [0, 16]]))
                eng_l = nc.vector if tail else nc.gpsimd
                eng_l.tensor_mul(
                    e2l[:],
                    ap(e2[:], 0, [[128, ct], [32, 4], [1, 16]]),
                    ap(r2[:], 0, [[8, ct], [2, 4], [0, 16]]))
                return e2h, e2l

            def pc_s4(t0, ct, e2h, e2l, tail=False):
                for gsub in range(ct // 4):
                    rep = prep.tile([P, 4, 4, 16, 16], f16, tag="rep")
                    nc.scalar.copy(
                        rep[:],
                        ap(e2h[:], 256 * gsub,
                           [[64, 4], [16, 4], [1, 16], [0, 16]]))
                    for t in range(4):
                        i = t0 + 4 * gsub + t
                        o_t = pout.tile([P, 4, 16, 16], f16, tag="o_t")
                        nc.vector.tensor_mul(
                            o_t[:],
                            ap(rep[:], 1024 * t, [[256, 4], [16, 16], [1, 16]]),
                            ap(e2l[:], 64 * (4 * gsub + t),
                               [[16, 4], [0, 16], [1, 16]]))
                        nc.scalar.dma_start(
                            o_v[i], o_t[:].rearrange("p a b c -> p (a b c)"))

            # ================= emission schedule =================
            # software-pipelined phase A; carry chain per 8-tile quarter;
            # phase-C chunk q stages spread over tiles 8q+9 .. 8q+15 so only
            # the last quarter's chunk runs after phase A ends.
            pending = {}  # c -> (stage, payload)

            def advance_chunk(key, tail=False):
                t0, ct = key
                st, payload = pending.get(key, (0, None))
                if st == 0:
                    pending[key] = (1, pc_s1(t0, ct))
                elif st == 1:
                    pending[key] = (2, pc_s2(t0, ct, payload))
                elif st == 2:
                    pending[key] = (3, pc_s3(t0, ct, payload, tail))
                elif st == 3:
                    pc_s4(t0, ct, *payload, tail=tail)
                    pending[key] = (4, None)

            sched = {}
            for q in range(3):
                for k in range(4):
                    sched.setdefault(4 * q + 5 + k, []).append((8 * q, 8))
            sched.setdefault(14, []).append((24, 4))
            sched.setdefault(15, []).append((24, 4))

            NU = NT // 2
            from collections import deque
            prevs = deque()
            for i in range(NU):
                prevs.append((i, phase_a1(i)))
                if i >= 1:
                    j, payload = prevs.popleft()
                    phase_a2(j, *payload)
                if i >= 4 and i % 4 == 0:
                    chain_q(8 * (i // 4 - 1))
                if i == 14:
                    chain_q(24, 4)
                for c in sched.get(i, []):
                    advance_chunk(c)
            while prevs:
                j, payload = prevs.popleft()
                phase_a2(j, *payload)
            chain_q(28, 4)
            for q in range(3):  # flush any unfinished in-loop chunks
                while pending.get((8 * q, 8), (0, None))[0] < 4:
                    advance_chunk((8 * q, 8))
            # tail: finish (24,4), run (28,4), 2-wide
            for _ in range(4):
                advance_chunk((28, 4), tail=True)
                if pending.get((24, 4), (0, None))[0] < 4:
                    advance_chunk((24, 4), tail=True)

    nc.compile()
    return nc


def _get_nc():
    global _BUILT
    if _BUILT is None:
        _BUILT = _build()
    return _BUILT


def kernel(a, b, add_table=None, carry_table=None, b2n=None, n2b=None, **_kw):
    from concourse.bass_utils import run_bass_kernel_spmd

    a = np.ascontiguousarray(np.asarray(a, dtype=np.float32))
    b = np.ascontiguousarray(np.asarray(b, dtype=np.float32))
    nc = _get_nc()
    in_maps = [
        {"a": a[i * B_CORE:(i + 1) * B_CORE], "b": b[i * B_CORE:(i + 1) * B_CORE]}
        for i in range(N_CORES)
    ]
    res = run_bass_kernel_spmd(nc, in_maps, core_ids=list(range(N_CORES)))
    out = np.concatenate([r["out"] for r in res.results], axis=0)
    return out.astype(np.float32)
